# revision 25
# baseline (speedup 1.0000x reference)
"""Bahdanau attention weights kernel for 8 Trainium2 NeuronCores.

Reference computation (per full input):
    proj_enc = encoder_output @ W1_w + W1_b            # [B,S,U]
    proj_h   = last_layer_h_n @ W2_w + W2_b            # [B,1,U]
    score    = tanh(proj_enc + proj_h) @ V_w + V_b     # [B,S,1]
    out      = softmax(score, axis=1)                  # [B,S,1]

Sharding: data-parallel over batch. Each of the 8 cores gets B/8 batches;
weights are replicated; softmax is over the local sequence axis, so no
cross-core communication is needed.

Per-core layout strategy ("scheme B"): keep U on partitions.
  - W1 [h,u] is the matmul stationary operand in its natural layout.
  - X^T tiles ([h, t]) are produced with PE transpose-mode matmuls.
  - Main matmuls compute proj^T [u=128, t=512] in PSUM, accumulated over
    8 h-blocks, using float32r (full fp32 precision at bf16-rate for
    moving dim >= 256).
  - tanh runs on the scalar engine reading PSUM, with the combined bias
    (W1_b + W2_b + h_n @ W2)[u] as the per-partition bias operand.
  - The V contraction is a matmul with lhsT = V[u-block] ([128,1]),
    accumulating score^T [1, t] over the 8 u-blocks.
  - Softmax per batch is free-dim reduce / Exp(accum_out) / reciprocal /
    tensor_scalar on the [1, 2048] score row.
"""

import sys

for _p in ("/opt/trn_rl_repo", "/root/.axon_site/_ro/trn_rl_repo"):
    if _p not in sys.path:
        sys.path.append(_p)

import numpy as np

import concourse.bacc as bacc
import concourse.tile as tile
from concourse import mybir
from concourse.masks import make_identity

F32 = mybir.dt.float32
F32R = mybir.dt.float32r

B, S, H, U = 32, 2048, 1024, 1024
N_CORES = 8
B_LOCAL = B // N_CORES  # 4
P = 128
T_GROUP = 512  # tokens per group (matmul moving dim)


def build_kernel(b_local=B_LOCAL, s=S, h=H, u=U, x_bf16=False):
    """Build the per-core Bass program. All shape params must keep:
    s*b_local % T_GROUP == 0, s % T_GROUP == 0, h % 128 == 0, u % 128 == 0.

    x_bf16: convert X and W1 to bf16 before the PE. Halves the PE transpose
    cost (1 cycle/row instead of 2) at a ~0.3% relative-error cost."""
    BF16 = mybir.dt.bfloat16
    nc = bacc.Bacc()

    n_tok = b_local * s
    n_groups = n_tok // T_GROUP
    groups_per_batch = s // T_GROUP
    HB = h // P  # h blocks
    UB = u // P  # u blocks
    TSUB = T_GROUP // P  # 128-token sub-tiles per group

    enc = nc.dram_tensor("encoder_output", [n_tok, h], F32, kind="ExternalInput")
    hn = nc.dram_tensor("last_layer_h_n", [b_local, h], F32, kind="ExternalInput")
    w1 = nc.dram_tensor("W1_w", [h, u], F32, kind="ExternalInput")
    b1 = nc.dram_tensor("W1_b", [u], F32, kind="ExternalInput")
    w2 = nc.dram_tensor("W2_w", [h, u], F32, kind="ExternalInput")
    b2 = nc.dram_tensor("W2_b", [u], F32, kind="ExternalInput")
    vw = nc.dram_tensor("V_w", [u, 1], F32, kind="ExternalInput")
    vb = nc.dram_tensor("V_b", [1], F32, kind="ExternalInput")
    out = nc.dram_tensor("out", [b_local, s], F32, kind="ExternalOutput")

    enc_v = enc.ap().rearrange("(g i p) h -> g i p h", i=TSUB, p=P)
    w1_v = w1.ap().rearrange("(hb p) u -> hb p u", p=P)
    w2_v = w2.ap().rearrange("(hb p) u -> hb p u", p=P)

    with tile.TileContext(nc) as tc:
        with (
            tc.tile_pool(name="consts", bufs=1) as consts,
            tc.tile_pool(name="wpool", bufs=1) as wpool,
            tc.tile_pool(name="xpool", bufs=3 * TSUB) as xpool,
            tc.tile_pool(name="xtpool", bufs=3) as xtpool,
            tc.tile_pool(name="thpool", bufs=3) as thpool,
            tc.tile_pool(name="scpool", bufs=2) as scpool,
            tc.tile_pool(name="smpool", bufs=2) as smpool,
            tc.tile_pool(name="pst", bufs=2, space="PSUM") as pst,
            tc.tile_pool(name="psu", bufs=2, space="PSUM") as psu,
            tc.tile_pool(name="pssc", bufs=2, space="PSUM") as pssc,
            tc.tile_pool(name="psb", bufs=2, space="PSUM") as psb,
        ):
            # ---- constants -------------------------------------------------
            ident = consts.tile([P, P], F32)
            make_identity(nc, ident)
            if x_bf16:
                ident16 = consts.tile([P, P], BF16)
                nc.vector.tensor_copy(ident16, ident)

            # prefetch the first groups' X tiles ahead of the weight DMAs so
            # the PE has transpose work during the weight-load phase
            PREFETCH = 2
            x_pending = {}

            def issue_x(g):
                tiles = []
                for i in range(TSUB):
                    xt = xpool.tile([P, h], F32, tag="x")
                    nc.sync.dma_start(out=xt, in_=enc_v[g, i])
                    tiles.append(xt)
                x_pending[g] = tiles

            issue_x(0)
            if n_groups > 1:
                issue_x(1)

            # V in [u_p, u_blk] layout
            v_f32 = consts.tile([P, UB], F32)
            nc.sync.dma_start(
                out=v_f32, in_=vw.ap().rearrange("(ub p) one -> p (ub one)", p=P)
            )
            if x_bf16:
                v_sb = consts.tile([P, UB], BF16)
                nc.vector.tensor_copy(v_sb, v_f32)
            else:
                # fp32r matmul operands must be written as fp32r (rounded)
                v_sb = consts.tile([P, UB], F32R)
                nc.vector.tensor_copy(v_sb, v_f32)
            vb_sb = consts.tile([1, 1], F32)
            nc.sync.dma_start(out=vb_sb, in_=vb.ap().rearrange("(a b) -> a b", a=1))

            # W1_b + W2_b in [u_p, u_blk] layout
            b1_sb = consts.tile([P, UB], F32)
            nc.sync.dma_start(out=b1_sb, in_=b1.ap().rearrange("(ub p) -> p ub", p=P))
            b2_sb = consts.tile([P, UB], F32)
            nc.sync.dma_start(out=b2_sb, in_=b2.ap().rearrange("(ub p) -> p ub", p=P))
            b12_sb = consts.tile([P, UB], F32)
            nc.vector.tensor_add(b12_sb, b1_sb, b2_sb)

            # h_n natural [b_local, h]
            hn_sb = consts.tile([b_local, h], F32)
            nc.sync.dma_start(out=hn_sb, in_=hn.ap())

            # W2 tiles go through the x pool's slots (transient: only the
            # bias precompute reads them, then x loads recycle the bufs).
            # W1/W2 h-tiles are interleaved so the bias precompute and the
            # first main matmuls become ready at about the same time.
            # fp32r/bf16 matmul operands must be produced by a rounding
            # compute op, so W1 goes DMA -> staging -> convert
            w2_sb = []
            w1_sb = []
            w1_dt = BF16 if x_bf16 else F32R
            with tc.tile_pool(name="w1stage", bufs=2) as w1stage:
                for hb in range(HB):
                    t = xpool.tile([P, u], F32, tag="x")
                    nc.sync.dma_start(out=t, in_=w2_v[hb])
                    w2_sb.append(t)
                    stg = w1stage.tile([P, u], F32, tag="w1s")
                    nc.sync.dma_start(out=stg, in_=w1_v[hb])
                    tw = wpool.tile([P, u], w1_dt, tag=f"w1b_{hb}")
                    nc.vector.tensor_copy(tw, stg)
                    w1_sb.append(tw)

            # ---- bias precompute: bias[u, b] = h_n @ W2 + (b1 + b2) --------
            # transpose h_n -> [h, b_local] blocks
            hnT = consts.tile([P, HB, b_local], F32)
            for hb in range(HB):
                ps = psb.tile([P, b_local], F32, tag="bp")
                nc.tensor.transpose(ps, hn_sb[:, hb * P : (hb + 1) * P],
                                    ident[:b_local, :b_local])
                nc.vector.tensor_copy(hnT[:, hb, :], ps)

            bias_sb = consts.tile([P, UB, b_local], F32)
            for ub in range(UB):
                ps = psb.tile([P, b_local], F32, tag="bp")
                for hb in range(HB):
                    nc.tensor.matmul(
                        ps,
                        lhsT=w2_sb[hb][:, ub * P : (ub + 1) * P],
                        rhs=hnT[:, hb, :],
                        start=(hb == 0),
                        stop=(hb == HB - 1),
                    )
                nc.scalar.activation(
                    bias_sb[:, ub, :], ps,
                    mybir.ActivationFunctionType.Identity,
                    bias=b12_sb[:, ub : ub + 1],
                )

            # ---- main loop over token groups ------------------------------
            sc_row = None
            for g in range(n_groups):
                b = g // groups_per_batch
                gi = g % groups_per_batch

                if g + PREFETCH < n_groups:
                    issue_x(g + PREFETCH)
                x_tiles = x_pending.pop(g)

                if x_bf16:
                    # f32 -> bf16 converts run on the otherwise-idle gpsimd
                    x16_tiles = []
                    for i in range(TSUB):
                        x16 = xpool.tile([P, h], BF16, tag="x16")
                        nc.gpsimd.tensor_copy(x16, x_tiles[i])
                        x16_tiles.append(x16)
                    x_tiles = x16_tiles

                # transpose to X^T [h=128, t=512] blocks
                xdt = BF16 if x_bf16 else F32
                xid = ident16 if x_bf16 else ident
                xT = xtpool.tile([P, HB, T_GROUP], BF16 if x_bf16 else F32R,
                                 tag="xT")
                for hb in range(HB):
                    ps = pst.tile([P, T_GROUP], xdt, tag="tp")
                    for i in range(TSUB):
                        nc.tensor.transpose(
                            ps[:, i * P : (i + 1) * P],
                            x_tiles[i][:, hb * P : (hb + 1) * P],
                            xid,
                        )
                    nc.vector.tensor_copy(xT[:, hb, :], ps)

                # proj^T[u, t] blocks, tanh, V contraction
                score_ps = pssc.tile([1, T_GROUP], F32, tag="sc")
                for ub in range(UB):
                    pu = psu.tile([P, T_GROUP], F32, tag="pu")
                    for hb in range(HB):
                        nc.tensor.matmul(
                            pu,
                            lhsT=w1_sb[hb][:, ub * P : (ub + 1) * P],
                            rhs=xT[:, hb, :],
                            start=(hb == 0),
                            stop=(hb == HB - 1),
                        )
                    th = thpool.tile([P, T_GROUP], BF16 if x_bf16 else F32R,
                                     tag="th")
                    nc.scalar.activation(
                        th, pu,
                        mybir.ActivationFunctionType.Tanh,
                        bias=bias_sb[:, ub, b : b + 1],
                    )
                    nc.tensor.matmul(
                        score_ps,
                        lhsT=v_sb[:, ub : ub + 1],
                        rhs=th,
                        start=(ub == 0),
                        stop=(ub == UB - 1),
                    )

                # score chunk -> per-batch score row (adds V_b)
                if gi == 0:
                    sc_row = scpool.tile([1, s], F32, tag="scrow")
                nc.scalar.activation(
                    sc_row[:, gi * T_GROUP : (gi + 1) * T_GROUP], score_ps,
                    mybir.ActivationFunctionType.Identity,
                    bias=vb_sb,
                )

                # batch complete -> softmax over s and write out
                if gi == groups_per_batch - 1:
                    # scores are bounded (|score| <= sum|V_w| + |V_b| < 17),
                    # so exp without max-subtraction is safe in fp32
                    ex = smpool.tile([1, s], F32, tag="ex")
                    esum = smpool.tile([1, 1], F32, tag="esum")
                    nc.scalar.activation(
                        ex, sc_row,
                        mybir.ActivationFunctionType.Exp,
                        accum_out=esum,
                    )
                    rec = smpool.tile([1, 1], F32, tag="rec")
                    nc.vector.reciprocal(rec, esum)
                    nc.vector.tensor_scalar_mul(ex, ex, rec)
                    nc.sync.dma_start(out=out.ap()[b : b + 1, :], in_=ex)

    nc.compile()
    return nc


def kernel(**inputs):
    from concourse.bass_utils import run_bass_kernel_spmd

    enc = np.ascontiguousarray(np.asarray(inputs["encoder_output"], dtype=np.float32))
    hn = np.ascontiguousarray(np.asarray(inputs["last_layer_h_n"], dtype=np.float32))
    w1 = np.ascontiguousarray(np.asarray(inputs["W1_w"], dtype=np.float32))
    b1 = np.ascontiguousarray(np.asarray(inputs["W1_b"], dtype=np.float32))
    w2 = np.ascontiguousarray(np.asarray(inputs["W2_w"], dtype=np.float32))
    b2 = np.ascontiguousarray(np.asarray(inputs["W2_b"], dtype=np.float32))
    vw = np.ascontiguousarray(np.asarray(inputs["V_w"], dtype=np.float32))
    vb = np.ascontiguousarray(np.asarray(inputs["V_b"], dtype=np.float32))

    nc = build_kernel()
    in_maps = []
    for c in range(N_CORES):
        sl = slice(c * B_LOCAL, (c + 1) * B_LOCAL)
        in_maps.append({
            "encoder_output": enc[sl].reshape(B_LOCAL * S, H),
            "last_layer_h_n": hn[sl],
            "W1_w": w1, "W1_b": b1, "W2_w": w2, "W2_b": b2,
            "V_w": vw, "V_b": vb,
        })
    res = run_bass_kernel_spmd(nc, in_maps, core_ids=list(range(N_CORES)))
    outs = [res.results[c]["out"].reshape(B_LOCAL, S, 1) for c in range(N_CORES)]
    return np.concatenate(outs, axis=0)


# revision 34
# speedup vs baseline: 1.1551x; 1.1551x over previous
"""Bahdanau attention weights kernel for 8 Trainium2 NeuronCores.

Reference computation (per full input):
    proj_enc = encoder_output @ W1_w + W1_b            # [B,S,U]
    proj_h   = last_layer_h_n @ W2_w + W2_b            # [B,1,U]
    score    = tanh(proj_enc + proj_h) @ V_w + V_b     # [B,S,1]
    out      = softmax(score, axis=1)                  # [B,S,1]

Sharding: data-parallel over batch. Each of the 8 cores gets B/8 batches;
weights are replicated; softmax is over the local sequence axis, so no
cross-core communication is needed.

Per-core layout strategy: keep U on partitions.
  - W1 [h,u] is the matmul stationary operand in its natural layout.
  - X^T tiles ([h, t]) are produced with PE transpose-mode matmuls.
  - Main matmuls compute proj^T [u=128, t=512] in PSUM, accumulated over
    8 h-blocks, in a low-precision compute dtype LP (bf16 by default;
    float32r keeps near-fp32 accuracy at the same PE rate).
  - tanh runs on the scalar engine reading PSUM, with the combined bias
    (W1_b + W2_b + h_n @ W2)[u] as the per-partition bias operand.
  - The V contraction packs four M=1 matmuls into distinct 32-partition
    column groups of one PSUM bank (they run concurrently on the PE),
    then one masked ones-matmul merges the four partial rows.
  - Softmax per batch is Exp(accum_out) / reciprocal / tensor_scalar on
    the [1, 2048] score row (scores are bounded, so no max subtraction).
"""

import sys

for _p in ("/opt/trn_rl_repo", "/root/.axon_site/_ro/trn_rl_repo"):
    if _p not in sys.path:
        sys.path.append(_p)

import numpy as np

import concourse.bacc as bacc
import concourse.tile as tile
from concourse import mybir
from concourse.masks import make_identity

F32 = mybir.dt.float32
F32R = mybir.dt.float32r
BF16 = mybir.dt.bfloat16

B, S, H, U = 32, 2048, 1024, 1024
N_CORES = 8
B_LOCAL = B // N_CORES  # 4
P = 128
T_GROUP = 512  # tokens per group (matmul moving dim)


def build_kernel(b_local=B_LOCAL, s=S, h=H, u=U, x_bf16=True):
    """Build the per-core Bass program. Shape params must keep:
    s % T_GROUP == 0, h % 128 == 0, u % 512 == 0, u/128 divisible by 4."""
    nc = bacc.Bacc()

    LP = BF16 if x_bf16 else F32R
    n_tok = b_local * s
    n_groups = n_tok // T_GROUP
    groups_per_batch = s // T_GROUP
    HB = h // P   # h blocks
    UB = u // P   # u blocks
    UH = u // T_GROUP  # 512-wide u halves (for the bias matmul)
    TSUB = T_GROUP // P
    QUAD = min(4, UB)  # V-matmuls packed per PSUM column-group set
    assert UB % QUAD == 0

    enc = nc.dram_tensor("encoder_output", [n_tok, h], F32, kind="ExternalInput")
    hn = nc.dram_tensor("last_layer_h_n", [b_local, h], F32, kind="ExternalInput")
    w1 = nc.dram_tensor("W1_w", [h, u], F32, kind="ExternalInput")
    b1 = nc.dram_tensor("W1_b", [u], F32, kind="ExternalInput")
    w2 = nc.dram_tensor("W2_w", [h, u], F32, kind="ExternalInput")
    b2 = nc.dram_tensor("W2_b", [u], F32, kind="ExternalInput")
    vw = nc.dram_tensor("V_w", [u, 1], F32, kind="ExternalInput")
    vb = nc.dram_tensor("V_b", [1], F32, kind="ExternalInput")
    out = nc.dram_tensor("out", [b_local, s], F32, kind="ExternalOutput")

    enc_v = enc.ap().rearrange("(g i p) h -> g i p h", i=TSUB, p=P)
    w1_v = w1.ap().rearrange("(hb p) u -> hb p u", p=P)
    w2_v = w2.ap().rearrange("(hb p) u -> hb p u", p=P)

    XBUFS = (3 if x_bf16 else 2) * TSUB
    XTBUFS = 3 if x_bf16 else 2

    with tile.TileContext(nc) as tc:
        with (
            tc.tile_pool(name="consts", bufs=1) as consts,
            tc.tile_pool(name="wpool", bufs=1) as wpool,
            tc.tile_pool(name="xpool", bufs=XBUFS) as xpool,
            tc.tile_pool(name="xtpool", bufs=XTBUFS) as xtpool,
            tc.tile_pool(name="thpool", bufs=3) as thpool,
            tc.tile_pool(name="scpool", bufs=2) as scpool,
            tc.tile_pool(name="smpool", bufs=2) as smpool,
            tc.tile_pool(name="pst", bufs=2, space="PSUM") as pst,
            tc.tile_pool(name="psu", bufs=2, space="PSUM") as psu,
            tc.tile_pool(name="pssc", bufs=2, space="PSUM") as pssc,
            tc.tile_pool(name="psmg", bufs=2, space="PSUM") as psmg,
        ):
            # ---- constants -------------------------------------------------
            ident = consts.tile([P, P], F32)
            make_identity(nc, ident)
            identL = consts.tile([P, P], LP)
            nc.vector.tensor_copy(identL, ident)

            # prefetch the first groups' X tiles ahead of the weight DMAs so
            # the PE has transpose work during the weight-load phase
            PREFETCH = 2
            x_pending = {}

            def issue_x(g):
                tiles = []
                for i in range(TSUB):
                    xt = xpool.tile([P, h], F32, tag="x")
                    nc.sync.dma_start(out=xt, in_=enc_v[g, i])
                    tiles.append(xt)
                x_pending[g] = tiles

            issue_x(0)
            if n_groups > 1:
                issue_x(1)

            # V in [u_p, u_blk] layout (rounded to LP for the V matmuls)
            v_f32 = consts.tile([P, UB], F32)
            nc.sync.dma_start(
                out=v_f32, in_=vw.ap().rearrange("(ub p) one -> p (ub one)", p=P)
            )
            v_sb = consts.tile([P, UB], LP)
            nc.vector.tensor_copy(v_sb, v_f32)
            vb_sb = consts.tile([1, 1], F32)
            nc.sync.dma_start(out=vb_sb, in_=vb.ap().rearrange("(a b) -> a b", a=1))

            # merge mask: 1.0 on partitions {0,32,64,96}, used to sum the
            # QUAD packed V-contraction rows with one matmul
            vmask = consts.tile([P, 1], LP)
            nc.vector.memset(vmask, 0.0)
            for j in range(QUAD):
                nc.vector.memset(vmask[32 * j : 32 * j + 1, :], 1.0)

            # W1_b + W2_b in [u_p, u_blk] layout
            b1_sb = consts.tile([P, UB], F32)
            nc.sync.dma_start(out=b1_sb, in_=b1.ap().rearrange("(ub p) -> p ub", p=P))
            b2_sb = consts.tile([P, UB], F32)
            nc.sync.dma_start(out=b2_sb, in_=b2.ap().rearrange("(ub p) -> p ub", p=P))
            b12_sb = consts.tile([P, UB], F32)
            nc.vector.tensor_add(b12_sb, b1_sb, b2_sb)

            # h_n natural [b_local, h], rounded to LP
            hn_f32 = consts.tile([b_local, h], F32)
            nc.sync.dma_start(out=hn_f32, in_=hn.ap())
            hn_sb = consts.tile([b_local, h], LP)
            nc.vector.tensor_copy(hn_sb, hn_f32)

            # W1 and W2 h-tiles, interleaved, staged f32 -> rounded LP.
            # Staging tiles ride the x pool's slots (transient).
            w1_sb = []
            w2_sb = []
            with tc.tile_pool(name="wstage", bufs=2) as wstage:
                for hb in range(HB):
                    stg2 = xpool.tile([P, u], F32, tag="x")
                    nc.sync.dma_start(out=stg2, in_=w2_v[hb])
                    t2 = xpool.tile([P, u], LP, tag="x16")
                    nc.vector.tensor_copy(t2, stg2)
                    w2_sb.append(t2)
                    stg1 = wstage.tile([P, u], F32, tag="w1s")
                    nc.sync.dma_start(out=stg1, in_=w1_v[hb])
                    t1 = wpool.tile([P, u], LP, tag=f"w1b_{hb}")
                    nc.vector.tensor_copy(t1, stg1)
                    w1_sb.append(t1)

            # transpose h_n -> hnT [h=128, b] blocks (LP)
            hnT = consts.tile([P, HB, b_local], LP)
            for hb in range(HB):
                ps = pst.tile([P, T_GROUP], LP, tag="tp")
                nc.tensor.transpose(
                    ps[:, :b_local], hn_sb[:, hb * P : (hb + 1) * P],
                    identL[:b_local, :b_local],
                )
                nc.vector.tensor_copy(hnT[:, hb, :], ps[:, :b_local])

            # ---- bias precompute: bias[u, b] = h_n @ W2 + (b1 + b2) --------
            # computed as [b, u] with W2 as the 512-wide moving operand,
            # then transposed back to [u, b] blocks
            bias_sb = consts.tile([P, UB, b_local], F32)
            for uh in range(UH):
                ps4 = pst.tile([P, T_GROUP], F32, tag="tp")
                for hb in range(HB):
                    nc.tensor.matmul(
                        ps4[:b_local, :],
                        lhsT=hnT[:, hb, :],
                        rhs=w2_sb[hb][:, uh * T_GROUP : (uh + 1) * T_GROUP],
                        start=(hb == 0),
                        stop=(hb == HB - 1),
                    )
                bstage = thpool.tile([b_local, T_GROUP], F32, tag="bstage")
                nc.vector.tensor_copy(bstage, ps4[:b_local, :])
                for i in range(TSUB):
                    ub = uh * TSUB + i
                    psb_t = pst.tile([P, T_GROUP], F32, tag="tp")
                    nc.tensor.transpose(
                        psb_t[:, :b_local],
                        bstage[:, i * P : (i + 1) * P],
                        ident[:b_local, :b_local],
                    )
                    nc.scalar.activation(
                        bias_sb[:, ub, :], psb_t[:, :b_local],
                        mybir.ActivationFunctionType.Identity,
                        bias=b12_sb[:, ub : ub + 1],
                    )

            # ---- main loop over token groups ------------------------------
            sc_row = None
            for g in range(n_groups):
                b = g // groups_per_batch
                gi = g % groups_per_batch

                if g + PREFETCH < n_groups:
                    issue_x(g + PREFETCH)
                x_tiles = x_pending.pop(g)

                # f32 -> LP rounding on the DVE
                xL_tiles = []
                for i in range(TSUB):
                    xL = xpool.tile([P, h], LP, tag="x16")
                    nc.vector.tensor_copy(xL, x_tiles[i])
                    xL_tiles.append(xL)

                # transpose to X^T [h=128, t=512] blocks
                xT = xtpool.tile([P, HB, T_GROUP], LP, tag="xT")
                for hb in range(HB):
                    ps = pst.tile([P, T_GROUP], LP, tag="tp")
                    for i in range(TSUB):
                        nc.tensor.transpose(
                            ps[:, i * P : (i + 1) * P],
                            xL_tiles[i][:, hb * P : (hb + 1) * P],
                            identL,
                        )
                    nc.vector.tensor_copy(xT[:, hb, :], ps)

                # proj^T[u, t] blocks + tanh
                score_q = pssc.tile([P, T_GROUP], F32, tag="sc")
                # rows outside the QUAD column-groups stay unwritten by the
                # packed matmuls; zero them so the masked merge reads zeros
                nc.vector.memset(score_q, 0.0)
                th_tiles = []
                for ub in range(UB):
                    pu = psu.tile([P, T_GROUP], F32, tag="pu")
                    for hb in range(HB):
                        nc.tensor.matmul(
                            pu,
                            lhsT=w1_sb[hb][:, ub * P : (ub + 1) * P],
                            rhs=xT[:, hb, :],
                            start=(hb == 0),
                            stop=(hb == HB - 1),
                        )
                    th = thpool.tile([P, T_GROUP], LP, tag="th", bufs=UB + 2)
                    nc.scalar.activation(
                        th, pu,
                        mybir.ActivationFunctionType.Tanh,
                        bias=bias_sb[:, ub, b : b + 1],
                    )
                    th_tiles.append(th)

                # packed V contraction, all back-to-back so each QUAD of
                # M=1 matmuls runs concurrently in distinct column groups
                for ub in range(UB):
                    j = ub % QUAD
                    q = ub // QUAD
                    nc.tensor.matmul(
                        score_q[32 * j : 32 * j + 1, :],
                        lhsT=v_sb[:, ub : ub + 1],
                        rhs=th_tiles[ub],
                        start=(q == 0),
                        stop=(q == UB // QUAD - 1),
                        skip_group_check=True,
                        tile_position=(0, 32 * j),
                    )

                # merge the QUAD partial rows: ones-mask matmul
                scm = thpool.tile([P, T_GROUP], LP, tag="scm")
                nc.vector.tensor_copy(scm, score_q)
                score_ps = psmg.tile([1, T_GROUP], F32, tag="mg")
                nc.tensor.matmul(score_ps, lhsT=vmask, rhs=scm)

                # score chunk -> exp incrementally per chunk (adds V_b).
                # scores are bounded (|score| <= sum|V_w| + |V_b| < 17), so
                # exp without max-subtraction is safe in fp32.
                if gi == 0:
                    sc_row = scpool.tile([1, s], F32, tag="scrow")
                    esums = smpool.tile([1, groups_per_batch], F32, tag="esums")
                nc.scalar.activation(
                    sc_row[:, gi * T_GROUP : (gi + 1) * T_GROUP], score_ps,
                    mybir.ActivationFunctionType.Exp,
                    bias=vb_sb,
                    accum_out=esums[:, gi : gi + 1],
                )

                # batch complete -> normalize and write out
                if gi == groups_per_batch - 1:
                    esum = smpool.tile([1, 1], F32, tag="esum")
                    nc.vector.tensor_reduce(
                        esum, esums, axis=mybir.AxisListType.X,
                        op=mybir.AluOpType.add,
                    )
                    rec = smpool.tile([1, 1], F32, tag="rec")
                    nc.vector.reciprocal(rec, esum)
                    nc.vector.tensor_scalar_mul(sc_row, sc_row, rec)
                    nc.sync.dma_start(out=out.ap()[b : b + 1, :], in_=sc_row)

    nc.compile()
    return nc


def kernel(**inputs):
    from concourse.bass_utils import run_bass_kernel_spmd

    enc = np.ascontiguousarray(np.asarray(inputs["encoder_output"], dtype=np.float32))
    hn = np.ascontiguousarray(np.asarray(inputs["last_layer_h_n"], dtype=np.float32))
    w1 = np.ascontiguousarray(np.asarray(inputs["W1_w"], dtype=np.float32))
    b1 = np.ascontiguousarray(np.asarray(inputs["W1_b"], dtype=np.float32))
    w2 = np.ascontiguousarray(np.asarray(inputs["W2_w"], dtype=np.float32))
    b2 = np.ascontiguousarray(np.asarray(inputs["W2_b"], dtype=np.float32))
    vw = np.ascontiguousarray(np.asarray(inputs["V_w"], dtype=np.float32))
    vb = np.ascontiguousarray(np.asarray(inputs["V_b"], dtype=np.float32))

    nc = build_kernel()
    in_maps = []
    for c in range(N_CORES):
        sl = slice(c * B_LOCAL, (c + 1) * B_LOCAL)
        in_maps.append({
            "encoder_output": enc[sl].reshape(B_LOCAL * S, H),
            "last_layer_h_n": hn[sl],
            "W1_w": w1, "W1_b": b1, "W2_w": w2, "W2_b": b2,
            "V_w": vw, "V_b": vb,
        })
    res = run_bass_kernel_spmd(nc, in_maps, core_ids=list(range(N_CORES)))
    outs = [res.results[c]["out"].reshape(B_LOCAL, S, 1) for c in range(N_CORES)]
    return np.concatenate(outs, axis=0)
